# revision 1
# baseline (speedup 1.0000x reference)
"""Trainium2 Bass kernel for nn_AffNet (affinity network).

Reference computation:
    X_emb = X @ W                               # [N, E]
    aff_h = (Z_h @ X_emb^T) / (|X_emb| |Z_h|)   # cosine, [H, N, N]
    aff   = max_h aff_h                          # [N, N]
    aff   = (aff + aff^T) / 2                    # symmetrize
    aff   = (aff + 1) / 2                        # [0, 1]
    aff   = aff ** beta

Device strategy (8 NeuronCores, cyclic block-rotation SPMD):
  The 16x16 grid of 512x512 blocks decomposes under the cyclic shift
  pi_c(b) = (b + c) mod 16 into 8 isomorphic templates: core c handles
  pairs {(c, c+d), (c+8, c+8+d) : d=1..7} + {(c, c+8)} and diagonals
  {c, c+8}.  All cores run ONE program over the fixed template; the
  host rotates the operand planes by c blocks per core and un-rotates
  during output assembly.

  Operands are fully SBUF-resident: X'^T and Z'^T (normalized fp16,
  x0.25 folded into Z') live in one [128, 5, 8192] tile (10MB/core,
  half the input DMA of per-slot tiles), loaded by 6 phased DMAs that
  are emitted lazily between slots so output DMAs on the same SP queue
  interleave with them.

  Per [128 x 256] job: 8 matmuls split by role into two 2-bank PSUM
  tiles -- tEVAC holds heads {2,3} of the Q side (lhsT=X rows,
  rhs=Z cols) and P side (lhsT=Z rows, rhs=X cols), tKEEP holds heads
  {0,1}.  ScalarE evacuates tEVAC with one contiguous fp32->fp16 copy;
  DVE folds tKEEP with that copy in ONE fused mixed tensor_max (the
  single PSUM operand the DVE port allows), yielding 4 pooled lanes
  {Q0v2, Q1v3, P0v2, P1v3} shipped fp16 via SP/HWDGE.  PSUM is fully
  double-buffered (2+2 banks x 2), so PE never stalls on evacuation.
  Diagonal slots run the Q side only (2 lanes).  The host finishes
  max(l0,l1)+max(l2,l3)+0.5, the mirror transposes, and beta.

  TRN2 constraints that shaped this: matmul PSUM output is fp32-only
  (no 16-bit accumulate), DVE tensor ops accept at most ONE PSUM
  operand (and run 2x only on all-16-bit packed operands), and GpSimd
  cannot run TensorTensor at all -- so the pooling tree must flow
  through exactly this ScalarE-copy + DVE-mixed-max structure.

  Cost-model balance per core: DVE ~153us (~99% busy in steady state,
  bottleneck), ScalarE ~134us, DMA ~122us, PE ~112us -> ~164us total
  (vs 205us baseline; measured rel err 2.4e-5).  Phase 0 is issued on
  two engine queues in parallel to shorten the fill; the program ends
  on a diagonal slot whose final half-chunk DMA is small, shortening
  the drain; small eV/eQ pools (3 bufs) throttle ScalarE run-ahead so
  its copies stay coupled to the consuming DVE ops.
"""

import os as _os

import numpy as np

N_NODES = 8192
N_FEATURES = 512
EMB = 128
N_HEADS = 4
EPS = 1e-6
N_CORES = 8
BLK = 512
N_BLK = N_NODES // BLK        # 16
M_CHUNK = 128                 # rows per matmul (PSUM partitions)
JW = 256                      # job column width
N_W = BLK // JW
N_PAIRS = 15
N_DIAG = 2

# tuning knobs (sim-searched; env-overridable for experiments)
BUF_MM = int(_os.environ.get("AFF_BUF_MM", "26"))
BUF_EP = int(_os.environ.get("AFF_BUF_EP", "3"))
BUF_EQ = int(_os.environ.get("AFF_BUF_EQ", "3"))
BUF_EQ0 = int(_os.environ.get("AFF_BUF_EQ0", "5"))
BUF_MMD = int(_os.environ.get("AFF_BUF_MMD", "6"))

# template slot order: diagonals and low blocks first so each slot's
# operand columns are resident by the time compute reaches it
SLOTS = (
    [(0, 0)] + [(0, d) for d in range(1, 8)]
    + [(0, 8)] + [(8, 8 + d) for d in range(1, 8)] + [(8, 8)]
)
# input DMA phases (column ranges of the resident planes)
PHASES = [(0, 256), (256, 512), (512, 1024), (1024, 2048), (2048, 3072),
          (3072, 4096), (4096, 5120), (5120, 6144), (6144, 7168),
          (7168, 8192)]

_CACHE = {}
LAST_RESULT = None


def _split_multi_waits(nc, limit=1):
    """The walrus build in this environment encodes at most one semaphore
    wait per instruction ("Too many sync wait commands" otherwise), while
    Tile attaches several. Hoist extra waits onto same-engine NOPs inserted
    immediately before the instruction (waits still execute before it)."""
    import concourse.mybir as mybir

    for f in nc.m.functions:
        for bb in f.blocks:
            il = bb.instructions  # live list backing the block
            idx = 0
            while idx < len(il):
                inst = il[idx]
                si = inst.sync_info
                waits = list(si.on_wait) if si is not None and si.on_wait else []
                if len(waits) > limit:
                    ups = list(si.on_update) if si.on_update else []
                    inst.sync_info = mybir.SyncInfo(
                        on_wait=waits[:limit], on_update=ups
                    )
                    eng = nc.engines[inst.engine]
                    pos = idx
                    for j in range(limit, len(waits), limit):
                        nbi = eng.nop()
                        ninst = nbi.ins
                        # nop() appended itself to the current bb; detach it
                        removed = False
                        for f2 in nc.m.functions:
                            for bb2 in f2.blocks:
                                l2 = bb2.instructions
                                if l2 and l2[-1].name == ninst.name:
                                    l2.pop()
                                    removed = True
                                    break
                            if removed:
                                break
                        assert removed, "could not detach helper nop"
                        ninst.sync_info = mybir.SyncInfo(
                            on_wait=waits[j : j + limit], on_update=[]
                        )
                        il.insert(pos, ninst)
                        pos += 1
                        idx += 1
                idx += 1


def _build_program():
    import concourse.bass as bass
    import concourse.mybir as mybir
    import concourse.tile as tile

    nc = bass.Bass("TRN2", target_bir_lowering=False, debug=False)

    f16 = mybir.dt.float16
    f32 = mybir.dt.float32

    planes = nc.dram_tensor("planes", [EMB, 1 + N_HEADS, N_NODES], f16,
                            kind="ExternalInput")
    # pair slots ship {maxQ', maxP'} per row-chunk; diagonal slots ship
    # maxQ' only.  Host adds the two halves (+0.5) and mirrors.
    outd = nc.dram_tensor("outd", [N_PAIRS, BLK, 4, BLK], f16,
                          kind="ExternalOutput")
    outdd = nc.dram_tensor("outdd", [N_DIAG, BLK, 2, BLK], f16,
                           kind="ExternalOutput")

    n_m = BLK // M_CHUNK  # 4 row chunks per block

    with tile.TileContext(nc) as tc:
        with (
            tc.tile_pool(name="weights", bufs=1) as wpool,
            tc.tile_pool(name="psum", bufs=1, space="PSUM") as ppool,
            tc.tile_pool(name="work", bufs=2) as spool,
        ):
            pt = wpool.tile([EMB, 1 + N_HEADS, N_NODES], f16, name="pt")
            xt = pt[:, 0]
            zt = [pt[:, 1 + h] for h in range(N_HEADS)]

            def load_phase(k):
                a, b = PHASES[k]
                nc.sync.dma_start(out=pt[:, :, a:b],
                                  in_=planes[:, :, a:b])

            # phase 0 split across two engine queues so the HWDGE issue
            # latencies overlap and the first job's operands land sooner
            a, b = PHASES[0]
            nc.scalar.dma_start(out=pt[:, 3:5, a:b],
                                in_=planes[:, 3:5, a:b])
            nc.sync.dma_start(out=pt[:, 0:3, a:b],
                              in_=planes[:, 0:3, a:b])
            load_phase(1)
            load_phase(2)
            load_phase(3)
            phase_next = [4]

            p_idx = 0
            d_idx = 0
            n_chunk = 0
            for si, (r, c) in enumerate(SLOTS):
                is_diag = r == c
                last_slot = si == len(SLOTS) - 1
                for m in range(n_m):
                    rs = slice(r * BLK + m * M_CHUNK,
                               r * BLK + (m + 1) * M_CHUNK)
                    if is_diag:
                        mmd = spool.tile([M_CHUNK, 2, BLK], f16, tag="mmd",
                                         bufs=4)
                        for w in range(N_W):
                            cs = slice(c * BLK + w * JW,
                                       c * BLK + (w + 1) * JW)
                            ws = slice(w * JW, (w + 1) * JW)
                            tE = ppool.tile([M_CHUNK, 4, JW], f32, tag="q",
                                            bufs=2)
                            tK = ppool.tile([M_CHUNK, 4, JW], f32, tag="p",
                                            bufs=2)
                            nc.tensor.matmul(tE[:, 0], xt[:, rs],
                                             zt[2][:, cs],
                                             start=True, stop=True)
                            nc.tensor.matmul(tE[:, 1], xt[:, rs],
                                             zt[3][:, cs],
                                             start=True, stop=True)
                            eV = spool.tile([M_CHUNK, 2, JW], f16,
                                            tag="eQ0" if d_idx == 0
                                            else "eQ",
                                            bufs=BUF_EQ0 if d_idx == 0
                                            else BUF_EQ)
                            nc.scalar.copy(eV, tE[:, 0:2])
                            nc.tensor.matmul(tK[:, 0], xt[:, rs],
                                             zt[0][:, cs],
                                             start=True, stop=True)
                            nc.tensor.matmul(tK[:, 1], xt[:, rs],
                                             zt[1][:, cs],
                                             start=True, stop=True)
                            nc.vector.tensor_max(mmd[:, :, ws],
                                                 tK[:, 0:2], eV)
                            if last_slot and m == n_m - 1:
                                nc.sync.dma_start(
                                    out=outdd[d_idx,
                                              m * M_CHUNK:(m + 1) * M_CHUNK,
                                              :, ws],
                                    in_=mmd[:, :, ws])
                        if not (last_slot and m == n_m - 1):
                            nc.sync.dma_start(
                                out=outdd[d_idx,
                                          m * M_CHUNK:(m + 1) * M_CHUNK,
                                          :, :],
                                in_=mmd)
                        continue
                    mm = spool.tile([M_CHUNK, 4, BLK], f16, tag="mm",
                                    bufs=BUF_MM)
                    for w in range(N_W):
                        cs = slice(c * BLK + w * JW, c * BLK + (w + 1) * JW)
                        ws = slice(w * JW, (w + 1) * JW)
                        # tEVAC holds heads {2,3} of both sides, evacuated
                        # by one contiguous ScalarE copy; tKEEP holds heads
                        # {0,1}, folded with the copy by ONE fused DVE
                        # mixed-max (single contiguous PSUM operand)
                        tE = ppool.tile([M_CHUNK, 4, JW], f32, tag="q",
                                        bufs=2)
                        tK = ppool.tile([M_CHUNK, 4, JW], f32, tag="p",
                                        bufs=2)
                        nc.tensor.matmul(tE[:, 0], xt[:, rs], zt[2][:, cs],
                                         start=True, stop=True)
                        nc.tensor.matmul(tE[:, 1], xt[:, rs], zt[3][:, cs],
                                         start=True, stop=True)
                        nc.tensor.matmul(tE[:, 2], zt[2][:, rs], xt[:, cs],
                                         start=True, stop=True)
                        nc.tensor.matmul(tE[:, 3], zt[3][:, rs], xt[:, cs],
                                         start=True, stop=True)
                        eV = spool.tile([M_CHUNK, 4, JW], f16, tag="eP",
                                        bufs=BUF_EP)
                        nc.scalar.copy(eV, tE)
                        nc.tensor.matmul(tK[:, 0], xt[:, rs], zt[0][:, cs],
                                         start=True, stop=True)
                        nc.tensor.matmul(tK[:, 1], xt[:, rs], zt[1][:, cs],
                                         start=True, stop=True)
                        nc.tensor.matmul(tK[:, 2], zt[0][:, rs], xt[:, cs],
                                         start=True, stop=True)
                        nc.tensor.matmul(tK[:, 3], zt[1][:, rs], xt[:, cs],
                                         start=True, stop=True)
                        nc.vector.tensor_max(mm[:, :, ws], tK, eV)
                        n_chunk += 1
                        if last_slot and m == n_m - 1:
                            # final chunk: ship each half as soon as its
                            # mixed-max lands to shorten the drain tail
                            nc.sync.dma_start(
                                out=outd[p_idx,
                                         m * M_CHUNK:(m + 1) * M_CHUNK,
                                         :, ws],
                                in_=mm[:, :, ws])
                    if not (last_slot and m == n_m - 1):
                        nc.sync.dma_start(
                            out=outd[p_idx,
                                     m * M_CHUNK:(m + 1) * M_CHUNK, :, :],
                            in_=mm)
                if is_diag:
                    d_idx += 1
                else:
                    p_idx += 1
                if phase_next[0] < len(PHASES):
                    load_phase(phase_next[0])
                    phase_next[0] += 1


    _split_multi_waits(nc)
    return nc


def _pairs_diags():
    pairs = [(r, c) for (r, c) in SLOTS if r != c]
    diags = [r for (r, c) in SLOTS if r == c]
    return pairs, diags


def kernel(X, W, Z, beta):
    global LAST_RESULT
    from concourse.bass_utils import run_bass_kernel_spmd

    X = np.asarray(X, dtype=np.float32)
    Wm = np.asarray(W, dtype=np.float32)
    Z = np.asarray(Z, dtype=np.float32)
    beta_f = float(np.asarray(beta))

    # Host: normalized, transposed fp16 operands (x0.25 folded into Z')
    X_emb = X @ Wm                                           # [N, E] fp32
    Xn = np.sqrt(np.sum(X_emb * X_emb, axis=-1))             # [N]
    Zn = np.sqrt(np.sum(Z * Z, axis=-1))                     # [H, N]
    Xp = X_emb / (Xn[:, None] + EPS)                         # [N, E]
    Zp = Z / (Zn[:, :, None] + EPS) * 0.25                   # [H, N, E]
    XpT = np.ascontiguousarray(Xp.T).astype(np.float16)      # [E, N]
    ZpT = np.ascontiguousarray(
        Zp.transpose(0, 2, 1)).astype(np.float16)            # [H, E, N]

    if "nc" not in _CACHE:
        _CACHE["nc"] = _build_program()
    nc = _CACHE["nc"]

    planes = np.concatenate([XpT[None], ZpT], axis=0)        # [5, E, N]
    planes = np.ascontiguousarray(planes.transpose(1, 0, 2))  # [E, 5, N]
    in_maps = []
    for cidx in range(N_CORES):
        sh = -cidx * BLK
        in_maps.append({
            "planes": np.ascontiguousarray(np.roll(planes, sh, axis=2)),
        })

    res = None
    for attempt in range(3):
        try:
            res = run_bass_kernel_spmd(nc, in_maps, list(range(N_CORES)))
            break
        except Exception:
            if attempt == 2:
                raise
    LAST_RESULT = res

    pairs, diags = _pairs_diags()
    outp = np.empty((N_NODES, N_NODES), dtype=np.float32)
    for cidx in range(N_CORES):
        outd = res.results[cidx]["outd"]    # [15,512,4,512] {Qa,Qb,Pa,Pb}
        outdd = res.results[cidx]["outdd"]  # [2,512,2,512] {Qa,Qb}
        for p, (r, c) in enumerate(pairs):
            R = (r + cidx) % N_BLK
            C = (c + cidx) % N_BLK
            S = np.maximum(outd[p, :, 0],
                           outd[p, :, 1]).astype(np.float32)
            S += np.maximum(outd[p, :, 2], outd[p, :, 3])
            S += np.float32(0.5)
            outp[R * BLK:(R + 1) * BLK, C * BLK:(C + 1) * BLK] = S
            outp[C * BLK:(C + 1) * BLK, R * BLK:(R + 1) * BLK] = S.T
        for d, r in enumerate(diags):
            R = (r + cidx) % N_BLK
            M = np.maximum(outdd[d, :, 0],
                           outdd[d, :, 1]).astype(np.float32)
            M += M.T
            M += np.float32(0.5)
            outp[R * BLK:(R + 1) * BLK, R * BLK:(R + 1) * BLK] = M

    if beta_f != 1.0:
        outp = np.power(outp, beta_f, dtype=np.float32)
    return outp



# revision 41
# speedup vs baseline: 1.0176x; 1.0176x over previous
"""Trainium2 Bass kernel for nn_AffNet (affinity network).

Reference computation:
    X_emb = X @ W                               # [N, E]
    aff_h = (Z_h @ X_emb^T) / (|X_emb| |Z_h|)   # cosine, [H, N, N]
    aff   = max_h aff_h                          # [N, N]
    aff   = (aff + aff^T) / 2                    # symmetrize
    aff   = (aff + 1) / 2                        # [0, 1]
    aff   = aff ** beta

Device strategy (8 NeuronCores, cyclic block-rotation SPMD):
  The 16x16 grid of 512x512 blocks decomposes under the cyclic shift
  pi_c(b) = (b + c) mod 16 into 8 isomorphic templates: core c handles
  pairs {(c, c+d), (c+8, c+8+d) : d=1..7} + {(c, c+8)} and diagonals
  {c, c+8}.  All cores run ONE program over the fixed template; the
  host rotates the operand planes by c blocks per core and un-rotates
  during output assembly.

  Operands are fully SBUF-resident: X'^T and Z'^T (normalized fp16,
  x0.25 folded into Z') live in one [128, 5, 8192] tile (10MB/core,
  half the input DMA of per-slot tiles), loaded by 6 phased DMAs that
  are emitted lazily between slots so output DMAs on the same SP queue
  interleave with them.

  Per [128 x 256] job: 8 matmuls split by role into two 2-bank PSUM
  tiles -- tEVAC holds heads {2,3} of the Q side (lhsT=X rows,
  rhs=Z cols) and P side (lhsT=Z rows, rhs=X cols), tKEEP holds heads
  {0,1}.  ScalarE evacuates tEVAC with one contiguous fp32->fp16 copy;
  DVE folds tKEEP with that copy in ONE fused mixed tensor_max (the
  single PSUM operand the DVE port allows), yielding 4 pooled lanes
  {Q0v2, Q1v3, P0v2, P1v3} shipped fp16 via SP/HWDGE.  PSUM is fully
  double-buffered (2+2 banks x 2), so PE never stalls on evacuation.
  Diagonal slots run the Q side only (2 lanes).  The host finishes
  max(l0,l1)+max(l2,l3)+0.5, the mirror transposes, and beta.

  TRN2 constraints that shaped this: matmul PSUM output is fp32-only
  (no 16-bit accumulate), DVE tensor ops accept at most ONE PSUM
  operand (and run 2x only on all-16-bit packed operands), and GpSimd
  cannot run TensorTensor at all -- so the pooling tree must flow
  through exactly this ScalarE-copy + DVE-mixed-max structure.

  Cost-model balance per core: DVE ~153us (~99% busy in steady state,
  bottleneck), ScalarE ~134us, DMA ~122us, PE ~112us -> ~164us total
  (vs 205us baseline; measured rel err 2.4e-5).  Phase 0 is issued on
  two engine queues in parallel to shorten the fill; the program ends
  on a diagonal slot whose final half-chunk DMA is small, shortening
  the drain; small eV/eQ pools (3 bufs) throttle ScalarE run-ahead so
  its copies stay coupled to the consuming DVE ops.
"""

import os as _os

import numpy as np

N_NODES = 8192
N_FEATURES = 512
EMB = 128
N_HEADS = 4
EPS = 1e-6
N_CORES = 8
BLK = 512
N_BLK = N_NODES // BLK        # 16
M_CHUNK = 128                 # rows per matmul (PSUM partitions)
JW = 256                      # job column width
N_W = BLK // JW
N_PAIRS = 15
N_DIAG = 2

# tuning knobs (sim-searched; env-overridable for experiments)
BUF_MM = int(_os.environ.get("AFF_BUF_MM", "24"))
BUF_EP = int(_os.environ.get("AFF_BUF_EP", "3"))
BUF_EQ = int(_os.environ.get("AFF_BUF_EQ", "3"))
BUF_EQ0 = int(_os.environ.get("AFF_BUF_EQ0", "5"))
BUF_MMD = int(_os.environ.get("AFF_BUF_MMD", "6"))
# R3 ("Act-heavy") jobs: ScalarE evacuates all 8 lanes unpooled and the
# host pools them, taking those jobs off the DVE entirely.  Picked so the
# two evacuation engines' busy times equalize (DVE 1.0417 ns/elem mixed vs
# ScalarE 0.8333): ~7% of the 120 pair jobs.
R3_EVERY = int(_os.environ.get("AFF_R3_EVERY", "0"))
R3_PHASE = int(_os.environ.get("AFF_R3_PHASE", "3"))
BUF_R3 = int(_os.environ.get("AFF_BUF_R3", "3"))


def _r3_chunks():
    """pair-job indices (over the 120 pair jobs, in issue order) that are
    evacuated Act-only (R3)."""
    if R3_EVERY <= 0:
        return []
    return [i for i in range(120) if i % R3_EVERY == R3_PHASE % R3_EVERY]

# template slot order: diagonals and low blocks first so each slot's
# operand columns are resident by the time compute reaches it
SLOTS = (
    [(0, 0)] + [(0, d) for d in range(1, 8)]
    + [(0, 8)] + [(8, 8 + d) for d in range(1, 8)]
)
# diag block 8's jobs are emitted one-at-a-time between pair jobs (their
# ScalarE-only evacuation replaces a pair eV copy beat, so the DVE loses
# those 8 jobs entirely); keys: pair-job index -> (m, w)
DIAG8_SCHED = {64: (0, 0), 71: (0, 1), 78: (1, 0), 85: (1, 1),
               92: (2, 0), 99: (2, 1), 106: (3, 0), 113: (3, 1)}
# input DMA phases (column ranges of the resident planes).  Third field:
# plane range -- cols 0:512 of the z planes live in the fill sliver, so
# the first two phases only load the x plane.
PHASES = [(128, 256, 1), (256, 512, 1), (512, 1024, 5), (1024, 2048, 5),
          (2048, 3072, 5), (3072, 4096, 5), (4096, 5120, 5),
          (5120, 6144, 5), (6144, 7168, 5), (7168, 8192, 5)]

_CACHE = {}
LAST_RESULT = None


def _split_multi_waits(nc, limit=1):
    """The walrus build in this environment encodes at most one semaphore
    wait per instruction ("Too many sync wait commands" otherwise), while
    Tile attaches several. Hoist extra waits onto same-engine NOPs inserted
    immediately before the instruction (waits still execute before it)."""
    import concourse.mybir as mybir

    for f in nc.m.functions:
        for bb in f.blocks:
            il = bb.instructions  # live list backing the block
            idx = 0
            while idx < len(il):
                inst = il[idx]
                si = inst.sync_info
                waits = list(si.on_wait) if si is not None and si.on_wait else []
                if len(waits) > limit:
                    ups = list(si.on_update) if si.on_update else []
                    inst.sync_info = mybir.SyncInfo(
                        on_wait=waits[:limit], on_update=ups
                    )
                    eng = nc.engines[inst.engine]
                    pos = idx
                    for j in range(limit, len(waits), limit):
                        nbi = eng.nop()
                        ninst = nbi.ins
                        # nop() appended itself to the current bb; detach it
                        removed = False
                        for f2 in nc.m.functions:
                            for bb2 in f2.blocks:
                                l2 = bb2.instructions
                                if l2 and l2[-1].name == ninst.name:
                                    l2.pop()
                                    removed = True
                                    break
                            if removed:
                                break
                        assert removed, "could not detach helper nop"
                        ninst.sync_info = mybir.SyncInfo(
                            on_wait=waits[j : j + limit], on_update=[]
                        )
                        il.insert(pos, ninst)
                        pos += 1
                        idx += 1
                idx += 1


def _build_program():
    import concourse.bass as bass
    import concourse.mybir as mybir
    import concourse.tile as tile

    nc = bass.Bass("TRN2", target_bir_lowering=False, debug=False)

    f16 = mybir.dt.float16
    f32 = mybir.dt.float32

    planes = nc.dram_tensor("planes", [EMB, 1 + N_HEADS, N_NODES], f16,
                            kind="ExternalInput")
    # pair slots ship {maxQ', maxP'} per row-chunk; diagonal slots ship
    # maxQ' only.  Host adds the two halves (+0.5) and mirrors.
    outd = nc.dram_tensor("outd", [N_PAIRS, BLK, 4, BLK], f16,
                          kind="ExternalOutput")
    outdd = nc.dram_tensor("outdd", [1, BLK, 2, BLK], f16,
                           kind="ExternalOutput")
    # diag block 8 ships raw 4-lane chunks (ScalarE-only evacuation)
    outdr = nc.dram_tensor("outdr", [BLK, 4, BLK], f16,
                           kind="ExternalOutput")
    n_r3 = len(_r3_chunks())
    # half-R3 jobs ship the raw P-side lanes {P0,P1,P2,P3} fp16; the host
    # pools those while the device ships the pooled Q half via outd.
    outr = None
    if n_r3:
        outr = nc.dram_tensor("outr", [n_r3, M_CHUNK, 4, JW], f16,
                              kind="ExternalOutput")
    # fill sliver: exactly the first diag m-chunk's operands, one small DMA
    # on the SP queue ahead of the phased loads so the first matmuls start
    # ~1.5us earlier.  Layout: [xt 0:128 | z2 0:512 | z3 0:512 | z0 0:512
    # | z1 0:512]  -> [EMB, 2176] fp16.
    sliver = nc.dram_tensor("sliver", [EMB, 2176], f16,
                            kind="ExternalInput")

    n_m = BLK // M_CHUNK  # 4 row chunks per block

    with tile.TileContext(nc) as tc:
        with (
            tc.tile_pool(name="weights", bufs=1) as wpool,
            tc.tile_pool(name="psum", bufs=1, space="PSUM") as ppool,
            tc.tile_pool(name="work", bufs=2) as spool,
        ):
            pt = wpool.tile([EMB, 1 + N_HEADS, N_NODES], f16, name="pt")
            xt = pt[:, 0]
            zt = [pt[:, 1 + h] for h in range(N_HEADS)]

            def load_phase(k):
                a, b, npl = PHASES[k]
                nc.sync.dma_start(out=pt[:, 0:npl, a:b],
                                  in_=planes[:, 0:npl, a:b])

            # fill slivers on the SP queue: z cols 0:512 + x cols 0:128,
            # split in two so the first job's operands (piece A) land
            # ~1us sooner; every z[0:512] read in the program comes from
            # this tile, so phases 0/1 only carry the x plane and the
            # whole head of the DMA pipeline shrinks by ~0.8 MB.
            # Layout: [x 0:128 | z2,z3,z0,z1 cols 0:256 | z2,z3,z0,z1
            # cols 256:512].
            sv = wpool.tile([EMB, 2176], f16, name="sv")
            nc.sync.dma_start(out=sv[:, 0:640], in_=sliver[:, 0:640])
            nc.scalar.dma_start(out=sv[:, 640:1152],
                                in_=sliver[:, 640:1152])
            nc.sync.dma_start(out=sv[:, 1152:2176],
                              in_=sliver[:, 1152:2176])
            nc.scalar.dma_start(out=pt[:, 0:1, 128:512],
                                in_=planes[:, 0:1, 128:512])
            load_phase(2)
            load_phase(3)
            phase_next = [4]

            zoff_a = {2: 128, 3: 384, 0: 640, 1: 896}
            zoff_b = {2: 1152, 3: 1408, 0: 1664, 1: 1920}

            def x_src(sl):
                if sl.stop <= 128:
                    return sv[:, sl.start:sl.stop]
                return xt[:, sl]

            def z_src(h, sl):
                if sl.stop <= 256:
                    return sv[:, zoff_a[h] + sl.start:zoff_a[h] + sl.stop]
                if sl.stop <= 512:
                    return sv[:, zoff_b[h] + sl.start - 256:
                              zoff_b[h] + sl.stop - 256]
                return zt[h][:, sl]

            r3_set = set(_r3_chunks())
            p_idx = 0
            d_idx = 0
            n_chunk = 0
            r3_idx = [0]
            for si, (r, c) in enumerate(SLOTS):
                is_diag = r == c
                last_slot = si == len(SLOTS) - 1
                for m in range(n_m):
                    rs = slice(r * BLK + m * M_CHUNK,
                               r * BLK + (m + 1) * M_CHUNK)
                    if is_diag:
                        mmd = spool.tile([M_CHUNK, 2, BLK], f16, tag="mmd",
                                         bufs=4)
                        for w in range(N_W):
                            cs = slice(c * BLK + w * JW,
                                       c * BLK + (w + 1) * JW)
                            ws = slice(w * JW, (w + 1) * JW)
                            xsrc = x_src(rs)
                            zsrc = {h: z_src(h, cs) for h in range(4)}
                            tE = ppool.tile([M_CHUNK, 4, JW], f32, tag="q",
                                            bufs=2)
                            tK = ppool.tile([M_CHUNK, 4, JW], f32, tag="p",
                                            bufs=2)
                            nc.tensor.matmul(tE[:, 0], xsrc,
                                             zsrc[2],
                                             start=True, stop=True)
                            nc.tensor.matmul(tE[:, 1], xsrc,
                                             zsrc[3],
                                             start=True, stop=True)
                            eV = spool.tile([M_CHUNK, 2, JW], f16,
                                            tag="eQ0" if d_idx == 0
                                            else "eQ",
                                            bufs=BUF_EQ0 if d_idx == 0
                                            else BUF_EQ)
                            nc.scalar.copy(eV, tE[:, 0:2])
                            nc.tensor.matmul(tK[:, 0], xsrc,
                                             zsrc[0],
                                             start=True, stop=True)
                            nc.tensor.matmul(tK[:, 1], xsrc,
                                             zsrc[1],
                                             start=True, stop=True)
                            nc.vector.tensor_max(mmd[:, :, ws],
                                                 tK[:, 0:2], eV)
                            if last_slot and m == n_m - 1:
                                nc.sync.dma_start(
                                    out=outdd[d_idx,
                                              m * M_CHUNK:(m + 1) * M_CHUNK,
                                              :, ws],
                                    in_=mmd[:, :, ws])
                        if not (last_slot and m == n_m - 1):
                            nc.sync.dma_start(
                                out=outdd[d_idx,
                                          m * M_CHUNK:(m + 1) * M_CHUNK,
                                          :, :],
                                in_=mmd)
                        continue
                    r3_flags = [(n_chunk + w) in r3_set for w in range(N_W)]
                    any_r3 = any(r3_flags)
                    mm = None
                    if not all(r3_flags):
                        mm = spool.tile([M_CHUNK, 4, BLK], f16, tag="mm",
                                        bufs=BUF_MM)
                    for w in range(N_W):
                        cs = slice(c * BLK + w * JW, c * BLK + (w + 1) * JW)
                        ws = slice(w * JW, (w + 1) * JW)
                        is_r3 = r3_flags[w]
                        # tEVAC holds heads {2,3} of both sides, evacuated
                        # by one contiguous ScalarE copy; tKEEP holds heads
                        # {0,1}.  R1 jobs: DVE folds tKEEP with the copy in
                        # ONE fused mixed tensor_max.  R3 jobs: ScalarE also
                        # evacuates tKEEP raw (no DVE work) and the host
                        # pools -- this equalizes the two evac engines.
                        tE = ppool.tile([M_CHUNK, 4, JW], f32, tag="q",
                                        bufs=2)
                        tK = ppool.tile([M_CHUNK, 4, JW], f32, tag="p",
                                        bufs=2)
                        nc.tensor.matmul(tE[:, 0], x_src(rs), z_src(2, cs),
                                         start=True, stop=True)
                        nc.tensor.matmul(tE[:, 1], x_src(rs), z_src(3, cs),
                                         start=True, stop=True)
                        nc.tensor.matmul(tE[:, 2], z_src(2, rs), x_src(cs),
                                         start=True, stop=True)
                        nc.tensor.matmul(tE[:, 3], z_src(3, rs), x_src(cs),
                                         start=True, stop=True)
                        eV = spool.tile([M_CHUNK, 4, JW], f16, tag="eP",
                                        bufs=BUF_EP)
                        nc.scalar.copy(eV, tE)
                        nc.tensor.matmul(tK[:, 0], x_src(rs), z_src(0, cs),
                                         start=True, stop=True)
                        nc.tensor.matmul(tK[:, 1], x_src(rs), z_src(1, cs),
                                         start=True, stop=True)
                        nc.tensor.matmul(tK[:, 2], z_src(0, rs), x_src(cs),
                                         start=True, stop=True)
                        nc.tensor.matmul(tK[:, 3], z_src(1, rs), x_src(cs),
                                         start=True, stop=True)
                        if is_r3:
                            # half-R3: DVE pools only the Q half; ScalarE
                            # additionally evacuates raw {P0,P1}; raw
                            # {P2,P3} ship straight from eV.  Host pools P.
                            ePK = spool.tile([M_CHUNK, 2, JW], f16,
                                             tag="r3", bufs=BUF_R3)
                            nc.scalar.copy(ePK, tK[:, 2:4])
                            nc.vector.tensor_max(mm[:, 0:2, ws],
                                                 tK[:, 0:2], eV[:, 0:2])
                            nc.sync.dma_start(out=outr[r3_idx[0], :, 0:2],
                                              in_=ePK)
                            nc.sync.dma_start(out=outr[r3_idx[0], :, 2:4],
                                              in_=eV[:, 2:4])
                            r3_idx[0] += 1
                            nc.sync.dma_start(
                                out=outd[p_idx,
                                         m * M_CHUNK:(m + 1) * M_CHUNK,
                                         0:2, ws],
                                in_=mm[:, 0:2, ws])
                        else:
                            nc.vector.tensor_max(mm[:, :, ws], tK, eV)
                            if any_r3 or (last_slot and m == n_m - 1):
                                # ship each written half on its own (the
                                # other half is R3-partial / drain-tail)
                                nc.sync.dma_start(
                                    out=outd[p_idx,
                                             m * M_CHUNK:(m + 1) * M_CHUNK,
                                             :, ws],
                                    in_=mm[:, :, ws])
                        if n_chunk in DIAG8_SCHED:
                            # interleaved diag-8 job: 4 matmuls + one
                            # ScalarE copy of all 4 raw head lanes (same
                            # size as a pair eV beat -> no Act spike, and
                            # no DVE work at all); host pools + mirrors.
                            dm, dw = DIAG8_SCHED[n_chunk]
                            drs = slice(8 * BLK + dm * M_CHUNK,
                                        8 * BLK + (dm + 1) * M_CHUNK)
                            dcs = slice(8 * BLK + dw * JW,
                                        8 * BLK + (dw + 1) * JW)
                            tD = ppool.tile([M_CHUNK, 4, JW], f32, tag="q",
                                            bufs=2)
                            for h in range(4):
                                nc.tensor.matmul(tD[:, h], x_src(drs),
                                                 z_src(h, dcs),
                                                 start=True, stop=True)
                            md = spool.tile([M_CHUNK, 4, JW], f16,
                                            tag="md", bufs=4)
                            nc.scalar.copy(md, tD)
                            nc.sync.dma_start(
                                out=outdr[dm * M_CHUNK:(dm + 1) * M_CHUNK,
                                          :, dw * JW:(dw + 1) * JW],
                                in_=md)
                        n_chunk += 1
                    if not any_r3 and not (last_slot and m == n_m - 1):
                        nc.sync.dma_start(
                            out=outd[p_idx,
                                     m * M_CHUNK:(m + 1) * M_CHUNK, :, :],
                            in_=mm)
                if is_diag:
                    d_idx += 1
                else:
                    p_idx += 1
                if phase_next[0] < len(PHASES):
                    load_phase(phase_next[0])
                    phase_next[0] += 1


    _split_multi_waits(nc)
    return nc


def _pairs_diags():
    pairs = [(r, c) for (r, c) in SLOTS if r != c]
    diags = [r for (r, c) in SLOTS if r == c]
    return pairs, diags


def _r3_map():
    """For each R3 slot k (in outr order): (pair index p, m-chunk m, w)."""
    r3_set = set(_r3_chunks())
    out = []
    n_chunk = 0
    p_idx = 0
    for (r, c) in SLOTS:
        if r == c:
            continue
        for m in range(BLK // M_CHUNK):
            for w in range(N_W):
                if n_chunk in r3_set:
                    out.append((p_idx, m, w))
                n_chunk += 1
        p_idx += 1
    return out


def kernel(X, W, Z, beta):
    global LAST_RESULT
    from concourse.bass_utils import run_bass_kernel_spmd

    X = np.asarray(X, dtype=np.float32)
    Wm = np.asarray(W, dtype=np.float32)
    Z = np.asarray(Z, dtype=np.float32)
    beta_f = float(np.asarray(beta))

    # Host: normalized, transposed fp16 operands (x0.25 folded into Z')
    X_emb = X @ Wm                                           # [N, E] fp32
    Xn = np.sqrt(np.sum(X_emb * X_emb, axis=-1))             # [N]
    Zn = np.sqrt(np.sum(Z * Z, axis=-1))                     # [H, N]
    Xp = X_emb / (Xn[:, None] + EPS)                         # [N, E]
    Zp = Z / (Zn[:, :, None] + EPS) * 0.25                   # [H, N, E]
    XpT = np.ascontiguousarray(Xp.T).astype(np.float16)      # [E, N]
    ZpT = np.ascontiguousarray(
        Zp.transpose(0, 2, 1)).astype(np.float16)            # [H, E, N]

    if "nc" not in _CACHE:
        _CACHE["nc"] = _build_program()
    nc = _CACHE["nc"]

    planes = np.concatenate([XpT[None], ZpT], axis=0)        # [5, E, N]
    planes = np.ascontiguousarray(planes.transpose(1, 0, 2))  # [E, 5, N]
    in_maps = []
    for cidx in range(N_CORES):
        sh = -cidx * BLK
        pr = np.ascontiguousarray(np.roll(planes, sh, axis=2))
        sliver = np.concatenate(
            [pr[:, 0, 0:128],
             pr[:, 3, 0:256], pr[:, 4, 0:256],
             pr[:, 1, 0:256], pr[:, 2, 0:256],
             pr[:, 3, 256:512], pr[:, 4, 256:512],
             pr[:, 1, 256:512], pr[:, 2, 256:512]], axis=1)
        in_maps.append({
            "planes": pr,
            "sliver": np.ascontiguousarray(sliver),
        })

    res = None
    for attempt in range(3):
        try:
            res = run_bass_kernel_spmd(nc, in_maps, list(range(N_CORES)))
            break
        except Exception:
            if attempt == 2:
                raise
    LAST_RESULT = res

    pairs, diags = _pairs_diags()
    r3_by_p = {}
    for k, (p, m, w) in enumerate(_r3_map()):
        r3_by_p.setdefault(p, []).append((m, w, k))
    outp = np.empty((N_NODES, N_NODES), dtype=np.float32)
    for cidx in range(N_CORES):
        outd = res.results[cidx]["outd"]    # [15,512,4,512] {Qa,Qb,Pa,Pb}
        outdd = res.results[cidx]["outdd"]  # [2,512,2,512] {Qa,Qb}
        outr = res.results[cidx].get("outr")  # [n_r3,128,4,256] raw lanes
        for p, (r, c) in enumerate(pairs):
            R = (r + cidx) % N_BLK
            C = (c + cidx) % N_BLK
            S = np.maximum(outd[p, :, 0],
                           outd[p, :, 1]).astype(np.float32)
            S += np.maximum(outd[p, :, 2], outd[p, :, 3])
            S += np.float32(0.5)
            for m, w, k in r3_by_p.get(p, ()):
                # half-R3: outd lanes {0,1} hold the pooled Q half; outr
                # holds raw P lanes {P0,P1,P2,P3}; pool P on host
                ms = slice(m * M_CHUNK, (m + 1) * M_CHUNK)
                ws_ = slice(w * JW, (w + 1) * JW)
                q = np.maximum(outd[p, ms, 0, ws_],
                               outd[p, ms, 1, ws_]).astype(np.float32)
                pp_ = np.max(outr[k].astype(np.float32), axis=1)
                S[ms, ws_] = q + pp_ + np.float32(0.5)
            outp[R * BLK:(R + 1) * BLK, C * BLK:(C + 1) * BLK] = S
            outp[C * BLK:(C + 1) * BLK, R * BLK:(R + 1) * BLK] = S.T
        for d, r in enumerate(diags):
            R = (r + cidx) % N_BLK
            M = np.maximum(outdd[d, :, 0],
                           outdd[d, :, 1]).astype(np.float32)
            M += M.T
            M += np.float32(0.5)
            outp[R * BLK:(R + 1) * BLK, R * BLK:(R + 1) * BLK] = M
        # diag block 8: raw 4-lane chunks, pooled on host
        outdr = res.results[cidx]["outdr"]   # [512, 4, 512]
        R = (8 + cidx) % N_BLK
        M = np.max(outdr.astype(np.float32), axis=1)
        M += M.T
        M += np.float32(0.5)
        outp[R * BLK:(R + 1) * BLK, R * BLK:(R + 1) * BLK] = M

    if beta_f != 1.0:
        outp = np.power(outp, beta_f, dtype=np.float32)
    return outp

